# revision 20
# baseline (speedup 1.0000x reference)
"""Masked max-pool over span axis (MaxSpanRepr) on 8 Trainium2 cores.

Computation: out[b, l, d] = max_s( mask[b, s] ? spans[b, l, s, d] : -1e10 )
  spans          [2048, 13, 4, 1024] f32
  attention_mask [2048, 4] int32
  out            [2048, 13, 1024] f32

Strategy: data-parallel over batch, 256 examples per core; examples are
dealt to cores round-robin PER MASK PATTERN so every core has near-equal
class sizes (the shared NEFF sizes each class block by the max across
cores - balance minimizes padding). The 2e-2 rel-err budget admits bf16:
the host pre-rounds spans to bf16 (halving device read bytes), the
device computes and stores bf16, and the host upcasts on the way out
(max amplifies no error; total rel err ~2^-9).

Per core the spans shard is a [13312 x 2KB] chunk table (chunk r*4+s
for row r=(b,l)); row r needs the max over its valid chunks. The mask
pattern of a row has 1-2 maximal RUNS of consecutive valid s, so a row
needs 1-2 gather descriptors (avg 1.25) instead of one per chunk.
SWDGE descriptor generation on GpSimd costs ~2.5us fixed per gather
instruction + ~8ns per descriptor, so both instructions and descriptors
are scarce: rows are sorted by run-shape class ((4),(3),(1+2),(2),
(1+1),(1)) and each (class, run-slot) issues ONE dma_gather covering
the whole class block (elem_size=len*D, elem_step=D). The per-core
real count rides in num_idxs_reg, loaded from SBUF at runtime: the
decode-side ring reservation and the Q7 trailing-negative trim then
agree exactly (they MUST agree at 128-index granularity or the
descriptor ring desyncs and the device hangs), so per-core pad entries
(-1) cost neither descriptors nor bytes.

Per tile a small tensor_tensor max-tree (bf16 2x DVE mode) reduces to
[128, D], stored DENSELY in sorted order via HWDGE (nc.sync/nc.scalar
alternating) - no GpSimd scatter descriptors. k=0 rows never touch the
device: the host writes the bf16 -1e10 fill directly. The host
inverse-permutes rows while unsharding (it already owns the sort) and
upcasts bf16->f32. NEFF structure depends only on per-class tile
counts; cached per structure.
"""

import math

import numpy as np

import concourse.bass as bass
import concourse.mybir as mybir
from concourse.ap import AP
from concourse.bass_utils import run_bass_kernel_spmd
from concourse.library_overlay import lower_extended_insts
from concourse.tile import TileContext

B, L, S, D = 2048, 13, 4, 1024
N_CORES = 8
B_SH = B // N_CORES              # 256 examples per core
ROWS = B_SH * L                  # 3328 (b,l) rows per core
N_CHUNKS = ROWS * S              # 13312 2KB chunks per core
NEG_FILL = -1e10
PAD_IDX = -1

# Run structure per 4-bit mask pattern (bit s = mask[s] != 0), runs
# sorted by length ascending so shape == sorted run-length tuple.
_RUNS = {}
for _p in range(16):
    _bits = [(_p >> _s) & 1 for _s in range(S)]
    _rs, _s = [], 0
    while _s < S:
        if _bits[_s]:
            _l = 1
            while _s + _l < S and _bits[_s + _l]:
                _l += 1
            _rs.append((_s, _l))
            _s += _l
        else:
            _s += 1
    _RUNS[_p] = sorted(_rs, key=lambda r: r[1])
_SHAPE = {p: tuple(l for _, l in _RUNS[p]) for p in range(16)}
# device classes, biggest elements first (builds DMA backlog while the
# descriptor feed rate exceeds drain rate); () is host-handled
CLASSES = [(4,), (3,), (1, 2), (2,), (1, 1), (1,)]
_SID = np.array([CLASSES.index(_SHAPE[p]) if _SHAPE[p] else -1
                 for p in range(16)], np.int64)
_START = np.zeros((2, 16), np.int64)
for _p in range(16):
    for _j, (_st, _l) in enumerate(_RUNS[_p]):
        _START[_j, _p] = _st
# gather instruction list: (class, slot, run_len), fixed order
GATHERS = [(c, j, ln) for c, shape in enumerate(CLASSES)
           for j, ln in enumerate(shape)]

_NC_CACHE = {}


# The walrus build in this container supports a single sync-wait slot per
# instruction ("Too many sync wait commands" in setupSyncWait otherwise),
# while Tile freely attaches one wait per semaphore lane. Post-pass: for any
# instruction carrying N>1 waits, hoist N-1 of them onto NoOp instructions
# inserted just before it on the same engine (engines execute in order, so
# all waits still complete before the instruction runs).
def _split_multi_wait_instructions(nc):
    ctr = 0
    for fn in nc.m.functions:
        for blk in fn.blocks:
            insts = blk.instructions
            out = []
            changed = False
            for inst in insts:
                si = inst.sync_info
                waits = list(si.on_wait) if si is not None else []
                if len(waits) > 1:
                    changed = True
                    for w in waits[:-1]:
                        ctr += 1
                        nop = mybir.InstNoOp(
                            name=f"I-waitsplit-{ctr}", ins=[], outs=[])
                        nop.engine = inst.engine
                        nsi = mybir.SyncInfo(on_update=[], on_wait=[w])
                        nop.sync_info = nsi
                        out.append(nop)
                    si.on_wait = [waits[-1]]
                out.append(inst)
            if changed:
                blk.instructions = out


def _build_nc(T, PB):
    """T: per-class tile counts; PB: per-class full pair-blocks (256 rows
    each, guaranteed all-real on every core)."""
    key = (T, PB)
    if key in _NC_CACHE:
        return _NC_CACHE[key]
    from concourse import library_config

    tot_slots = 128 * sum(T)
    cols16 = sum(128 * T[c] for c, _, _ in GATHERS) // 16

    nc = bass.Bass(num_swdge_queues=3)
    bf16, i16 = mybir.dt.bfloat16, mybir.dt.int16
    i32 = mybir.dt.int32
    spans = nc.dram_tensor("spans", [N_CHUNKS, D], bf16,
                           kind="ExternalInput")
    gidx = nc.dram_tensor("gidx", [128, cols16], i16, kind="ExternalInput")
    ncnt = nc.dram_tensor("ncnt", [128, len(GATHERS)], i32,
                          kind="ExternalInput")
    out = nc.dram_tensor("out", [tot_slots, D], bf16, kind="ExternalOutput")

    sp = spans[:]

    def spans_view(run_len):
        # overlapping view [n, run_len*D] with row stride D: index unit is
        # one chunk, each gathered element spans run_len chunks
        if run_len == 1:
            return sp
        return AP(sp.tensor, sp.offset,
                  [[D, N_CHUNKS - run_len + 1], [1, run_len * D]])

    base_of = {}
    acc = 0
    for c in range(len(CLASSES)):
        base_of[c] = acc
        acc += 128 * T[c]

    with TileContext(nc) as tc:
        with (
            tc.tile_pool(name="constp", bufs=1) as const_pool,
            tc.tile_pool(name="outp", bufs=8) as out_pool,
        ):
            # counts first: the gpsimd register load is the longest
            # preamble dependency chain; gidx rides the other HWDGE ring
            ncnt_t = const_pool.tile([128, len(GATHERS)], i32)
            nc.sync.dma_start(out=ncnt_t[:], in_=ncnt[:])
            gidx_t = const_pool.tile([128, cols16], i16)
            nc.scalar.dma_start(out=gidx_t[:], in_=gidx[:])

            nc.gpsimd.load_library(library_config.mlp)
            counts = [
                nc.alloc_register(mybir.EngineType.Pool, f"cnt{gi}")
                for gi in range(len(GATHERS))]
            nc.gpsimd.reg_load(counts, ncnt_t[0:1, :])

            land = {}
            for c, shape in enumerate(CLASSES):
                for j, ln in enumerate(shape):
                    if T[c]:
                        land[(c, j)] = const_pool.tile(
                            [128, T[c], ln * D], bf16, name=f"land_{c}_{j}")

            # one gather instruction per (class, slot); per-core real
            # count in num_idxs_reg; queues rotate (each queue_num is a
            # distinct Q7 core pair -> concurrent descriptor generation;
            # the Q7 gather op supports queues 0-2)
            off16 = 0
            qn = 0
            for gi, (c, j, ln) in enumerate(GATHERS):
                if not T[c]:
                    continue
                n_idx = 128 * T[c]
                nc.gpsimd.dma_gather(
                    land[(c, j)][:], spans_view(ln),
                    gidx_t[:, off16:off16 + n_idx // 16],
                    n_idx, counts[gi], ln * D,
                    elem_step=(None if ln == 1 else D),
                    queue_num=qn)
                qn = (qn + 1) % 3
                off16 += n_idx // 16
            assert off16 == cols16, (off16, cols16)

            def tt_max(o, a_, b_):
                nc.vector.tensor_tensor(o, a_, b_, mybir.AluOpType.max)

            # reduce + dense store in class order; stores alternate
            # between the two HWDGE engines (sync / scalar)
            st_eng = [nc.sync, nc.scalar]
            st_i = 0

            def store(dst_rows, src):
                nonlocal st_i
                st_eng[st_i & 1].dma_start(out=dst_rows, in_=src)
                st_i += 1

            def reduce_groups(c, shape, gsl, o):
                """o <- max over the class's chunks for land group slice
                gsl (a slice of the group dim; [128, W, D]-shaped APs)."""
                if shape == (2,):
                    l0 = land[(c, 0)]
                    tt_max(o, l0[:, gsl, 0:D], l0[:, gsl, D:2 * D])
                elif shape == (3,):
                    l0 = land[(c, 0)]
                    tt_max(o, l0[:, gsl, 0:D], l0[:, gsl, D:2 * D])
                    tt_max(o, l0[:, gsl, 2 * D:3 * D], o)
                elif shape == (4,):
                    l0 = land[(c, 0)]
                    tt_max(o, l0[:, gsl, 0:D], l0[:, gsl, D:2 * D])
                    tt_max(o, l0[:, gsl, 2 * D:3 * D], o)
                    tt_max(o, l0[:, gsl, 3 * D:4 * D], o)
                elif shape == (1, 1):
                    tt_max(o, land[(c, 0)][:, gsl, :],
                           land[(c, 1)][:, gsl, :])
                elif shape == (1, 2):
                    l1 = land[(c, 1)]
                    tt_max(o, l1[:, gsl, 0:D], l1[:, gsl, D:2 * D])
                    tt_max(o, land[(c, 0)][:, gsl, :], o)
                else:
                    raise AssertionError(shape)

            def pair_rows_ap(r0):
                # DRAM rows [r0, r0+256) viewed [128, 2*D]: partition p
                # holds rows (r0+2p, r0+2p+1) -> 4KB descriptors
                v = out[r0:r0 + 256, :]
                return AP(v.tensor, v.offset, [[2 * D, 128], [1, 2 * D]])

            for c, shape in enumerate(CLASSES):
                # pair-blocks: groups (2b, 2b+1) hold rows base+b*256+2p+h
                # at (partition p, group-parity h): one [128, 2, D] reduce
                # and one [128, 2*D] store (4KB descriptors, 256 rows)
                for b in range(PB[c]):
                    r0 = base_of[c] + 256 * b
                    gsl = slice(2 * b, 2 * b + 2)
                    if shape == (1,):
                        src = land[(c, 0)][:, gsl, :]
                    else:
                        o = out_pool.tile([128, 2, D], bf16, tag="res")
                        reduce_groups(c, shape, gsl, o[:])
                        src = o[:]
                    store(pair_rows_ap(r0), src)
                # trailing single tiles (per-core pads land here)
                for t in range(2 * PB[c], T[c]):
                    r0 = base_of[c] + 128 * t
                    dst_rows = out[r0:r0 + 128, :]
                    if shape == (1,):
                        store(dst_rows, land[(c, 0)][:, t, :])
                        continue
                    o = out_pool.tile([128, 1, D], bf16, tag="res1")
                    reduce_groups(c, shape, slice(t, t + 1), o[:])
                    store(dst_rows, o[:, 0, :])

    lower_extended_insts(nc)
    _split_multi_wait_instructions(nc)
    _NC_CACHE[T] = nc
    return nc


def _f32_to_bf16_u16(a_f32):
    """Round-to-nearest-even f32 -> bf16 bit pattern (uint16)."""
    u = a_f32.view(np.uint32)
    return ((u + 0x8000 + ((u >> 16) & 1)) >> 16).astype(np.uint16)


def _assign_cores(mask):
    """Deal examples to cores round-robin per pattern: class sizes are
    balanced to +-1 example so the shared (maxed) NEFF pads least."""
    valid = (np.asarray(mask) != 0)
    pat_ex = (valid.astype(np.int64) * (1 << np.arange(S))).sum(1)  # [B]
    ex_of_core = [[] for _ in range(N_CORES)]
    rr = 0
    for p in range(16):
        for e in np.nonzero(pat_ex == p)[0]:
            ex_of_core[rr].append(int(e))
            rr = (rr + 1) % N_CORES
    # equalize totals to B_SH by moving surplus (keeps shard shapes equal)
    surplus = []
    for i in range(N_CORES):
        while len(ex_of_core[i]) > B_SH:
            surplus.append(ex_of_core[i].pop())
    for i in range(N_CORES):
        while len(ex_of_core[i]) < B_SH:
            ex_of_core[i].append(surplus.pop())
    return pat_ex, [np.array(e, np.int64) for e in ex_of_core]


def _core_tables(pat_rows, T, PB):
    """gidx stream + per-instruction counts + (order, slots) maps.
    pat_rows: [ROWS] pattern of each core-local row."""
    sid_rows = _SID[pat_rows]
    live = sid_rows >= 0
    order = np.argsort(
        np.where(live, sid_rows, 10 ** 6), kind="stable")  # k0 rows last
    sid_sorted = np.where(live[order], sid_rows[order], -1)

    per_class_idx = {}
    counts = np.empty(len(GATHERS), np.int32)
    slots = np.full(ROWS, -1, np.int64)
    base = 0
    row_pos = 0
    for c, shape in enumerate(CLASSES):
        rows_c = order[sid_sorted == c]
        n = len(rows_c)
        assert n <= 128 * T[c], (c, n, T[c])
        slots[row_pos:row_pos + n] = base + np.arange(n)
        row_pos += n
        for j in range(len(shape)):
            idx = np.full(128 * T[c], PAD_IDX, np.int64)
            idx[:n] = rows_c * S + _START[j, pat_rows[rows_c]]
            if n == 0 and T[c]:
                idx[0] = 0          # sentinel: >=1 real descriptor
            per_class_idx[(c, j)] = idx
        base += 128 * T[c]
    for gi, (c, j, ln) in enumerate(GATHERS):
        n = int((per_class_idx[(c, j)] >= 0).sum())
        counts[gi] = max(n, 1)

    # pair-region stream permutation: sorted row r = b*256 + 2p + h of
    # the class lands at stream position (2b+h)*128 + p; the trailing
    # singles region keeps the identity layout (r == position)
    segs = []
    for (c, j, ln) in GATHERS:
        idx_lin = per_class_idx[(c, j)]
        seg = idx_lin.copy()
        npair = 256 * PB[c]
        if npair:
            r = np.arange(npair)
            pos = (2 * (r // 256) + (r & 1)) * 128 + (r % 256) // 2
            seg[pos] = idx_lin[r]
        segs.append(seg)
    stream = np.concatenate(segs).astype(np.int16)
    cols16 = len(stream) // 16
    gidx16 = np.zeros((16, cols16), np.int16)
    ppos = np.arange(len(stream))
    gidx16[ppos % 16, ppos // 16] = stream
    gidx = np.tile(gidx16, (8, 1))                     # 8 Q7 cores
    ncnt = np.tile(counts[None, :], (128, 1))
    return gidx, ncnt, order, slots


def _make_all(spans, attention_mask):
    spans = np.asarray(spans)
    mask = np.asarray(attention_mask)
    assert spans.shape == (B, L, S, D), spans.shape
    assert mask.shape == (B, S), mask.shape

    pat_ex, ex_of_core = _assign_cores(mask)
    # per-core class sizes -> shared tile counts
    n_cls = np.zeros((N_CORES, len(CLASSES)), np.int64)
    pat_rows_core = []
    for i in range(N_CORES):
        pr = np.repeat(pat_ex[ex_of_core[i]], L)
        pat_rows_core.append(pr)
        sid = _SID[pr]
        n_cls[i] = np.bincount(sid[sid >= 0], minlength=len(CLASSES))
    T = tuple(int(math.ceil(int(n_cls[:, c].max()) / 128))
              for c in range(len(CLASSES)))
    # full 256-row pair-blocks, all-real on EVERY core (pads would fall
    # mid-stream there and desync the trimmed-count ring contract)
    PB = tuple(min(int(n_cls[:, c].min()) // 256, T[c] // 2)
               for c in range(len(CLASSES)))

    spans_f32 = np.ascontiguousarray(spans, dtype=np.float32)
    spans_rows = spans_f32.reshape(B * L, S * D)

    import ml_dtypes
    in_maps, unperm = [], []
    for i in range(N_CORES):
        gidx, ncnt, order, slots = _core_tables(pat_rows_core[i], T, PB)
        rows_g = (np.repeat(ex_of_core[i] * L, L)
                  + np.tile(np.arange(L), B_SH))      # global row ids
        sp_bf = _f32_to_bf16_u16(
            spans_rows[rows_g]).reshape(N_CHUNKS, D).view(ml_dtypes.bfloat16)
        in_maps.append({"spans": sp_bf, "gidx": gidx, "ncnt": ncnt})
        unperm.append((rows_g, order, slots))
    return T, PB, in_maps, unperm


def run(spans, attention_mask, **spmd_kwargs):
    """Run the device kernel; returns (full_output, BassKernelResults)."""
    T, PB, in_maps, unperm = _make_all(spans, attention_mask)
    nc = _build_nc(T, PB)
    res = run_bass_kernel_spmd(nc, in_maps, core_ids=list(range(N_CORES)),
                               **spmd_kwargs)
    neg_u16 = _f32_to_bf16_u16(np.float32([NEG_FILL]))[0]
    full_u16 = np.empty((B * L, D), np.uint16)
    for i in range(N_CORES):
        rows_g, order, slots = unperm[i]
        out_u16 = res.results[i]["out"].view(np.uint16)
        # sorted position p holds row order[p] in slot slots[p]
        live = slots >= 0
        rows_sorted = rows_g[order]
        full_u16[rows_sorted[live]] = out_u16[slots[live]]
        full_u16[rows_sorted[~live]] = neg_u16        # k=0 rows
    full = (full_u16.astype(np.uint32) << 16).view(np.float32)
    return full.reshape(B, L, D), res


def kernel(spans, attention_mask):
    full, _ = run(spans, attention_mask)
    return full


# revision 21
# speedup vs baseline: 1.1743x; 1.1743x over previous
"""Masked max-pool over span axis (MaxSpanRepr) on 8 Trainium2 cores.

Computation: out[b, l, d] = max_s( mask[b, s] ? spans[b, l, s, d] : -1e10 )
  spans          [2048, 13, 4, 1024] f32
  attention_mask [2048, 4] int32
  out            [2048, 13, 1024] f32

Strategy: data-parallel over batch, 256 examples per core; examples are
dealt to cores round-robin PER MASK PATTERN so every core has near-equal
class sizes (the shared NEFF sizes each class block by the max across
cores - balance minimizes padding). The 2e-2 rel-err budget admits bf16:
the host pre-rounds spans to bf16 (halving device read bytes), the
device computes and stores bf16, and the host upcasts on the way out
(max amplifies no error; total rel err ~2^-9).

Per core the spans shard is a [13312 x 2KB] chunk table (chunk r*4+s
for row r=(b,l)); row r needs the max over its valid chunks. The mask
pattern of a row has 1-2 maximal RUNS of consecutive valid s, so a row
needs 1-2 gather descriptors (avg 1.25) instead of one per chunk.
SWDGE descriptor generation on GpSimd costs ~2.5us fixed per gather
instruction + ~8ns per descriptor, so both instructions and descriptors
are scarce: rows are sorted by run-shape class ((4),(3),(1+2),(2),
(1+1),(1)) and each (class, run-slot) issues ONE dma_gather covering
the whole class block (elem_size=len*D, elem_step=D). The per-core
real count rides in num_idxs_reg, loaded from SBUF at runtime: the
decode-side ring reservation and the Q7 trailing-negative trim then
agree exactly (they MUST agree at 128-index granularity or the
descriptor ring desyncs and the device hangs), so per-core pad entries
(-1) cost neither descriptors nor bytes.

Per tile a small tensor_tensor max-tree (bf16 2x DVE mode) reduces to
[128, D], stored DENSELY in sorted order via HWDGE (nc.sync/nc.scalar
alternating) - no GpSimd scatter descriptors. k=0 rows never touch the
device: the host writes the bf16 -1e10 fill directly. The host
inverse-permutes rows while unsharding (it already owns the sort) and
upcasts bf16->f32. NEFF structure depends only on per-class tile
counts; cached per structure.
"""

import math

import numpy as np

import concourse.bass as bass
import concourse.mybir as mybir
from concourse.ap import AP
from concourse.bass_utils import run_bass_kernel_spmd
from concourse.library_overlay import lower_extended_insts
from concourse.tile import TileContext

B, L, S, D = 2048, 13, 4, 1024
N_CORES = 8
B_SH = B // N_CORES              # 256 examples per core
ROWS = B_SH * L                  # 3328 (b,l) rows per core
N_CHUNKS = ROWS * S              # 13312 2KB chunks per core
NEG_FILL = -1e10
PAD_IDX = -1

# Run structure per 4-bit mask pattern (bit s = mask[s] != 0), runs
# sorted by length ascending so shape == sorted run-length tuple.
_RUNS = {}
for _p in range(16):
    _bits = [(_p >> _s) & 1 for _s in range(S)]
    _rs, _s = [], 0
    while _s < S:
        if _bits[_s]:
            _l = 1
            while _s + _l < S and _bits[_s + _l]:
                _l += 1
            _rs.append((_s, _l))
            _s += _l
        else:
            _s += 1
    _RUNS[_p] = sorted(_rs, key=lambda r: r[1])
_SHAPE = {p: tuple(l for _, l in _RUNS[p]) for p in range(16)}
# device classes, biggest elements first (builds DMA backlog while the
# descriptor feed rate exceeds drain rate); () is host-handled
CLASSES = [(4,), (3,), (1, 2), (2,), (1, 1), (1,)]
_SID = np.array([CLASSES.index(_SHAPE[p]) if _SHAPE[p] else -1
                 for p in range(16)], np.int64)
_START = np.zeros((2, 16), np.int64)
for _p in range(16):
    for _j, (_st, _l) in enumerate(_RUNS[_p]):
        _START[_j, _p] = _st
# gather instruction list: (class, slot, run_len), fixed order
GATHERS = [(c, j, ln) for c, shape in enumerate(CLASSES)
           for j, ln in enumerate(shape)]

_NC_CACHE = {}


# The walrus build in this container supports a single sync-wait slot per
# instruction ("Too many sync wait commands" in setupSyncWait otherwise),
# while Tile freely attaches one wait per semaphore lane. Post-pass: for any
# instruction carrying N>1 waits, hoist N-1 of them onto NoOp instructions
# inserted just before it on the same engine (engines execute in order, so
# all waits still complete before the instruction runs).
def _split_multi_wait_instructions(nc):
    ctr = 0
    for fn in nc.m.functions:
        for blk in fn.blocks:
            insts = blk.instructions
            out = []
            changed = False
            for inst in insts:
                si = inst.sync_info
                waits = list(si.on_wait) if si is not None else []
                if len(waits) > 1:
                    changed = True
                    for w in waits[:-1]:
                        ctr += 1
                        nop = mybir.InstNoOp(
                            name=f"I-waitsplit-{ctr}", ins=[], outs=[])
                        nop.engine = inst.engine
                        nsi = mybir.SyncInfo(on_update=[], on_wait=[w])
                        nop.sync_info = nsi
                        out.append(nop)
                    si.on_wait = [waits[-1]]
                out.append(inst)
            if changed:
                blk.instructions = out


def _build_nc(T, PB):
    """T: per-class tile counts; PB: per-class full pair-blocks (256 rows
    each, guaranteed all-real on every core)."""
    key = (T, PB)
    if key in _NC_CACHE:
        return _NC_CACHE[key]
    from concourse import library_config

    tot_slots = 128 * sum(T)
    cols16 = sum(128 * T[c] for c, _, _ in GATHERS) // 16

    nc = bass.Bass(num_swdge_queues=3)
    bf16, i16 = mybir.dt.bfloat16, mybir.dt.int16
    i32 = mybir.dt.int32
    spans = nc.dram_tensor("spans", [N_CHUNKS, D], bf16,
                           kind="ExternalInput")
    gidx = nc.dram_tensor("gidx", [128, cols16], i16, kind="ExternalInput")
    ncnt = nc.dram_tensor("ncnt", [128, len(GATHERS)], i32,
                          kind="ExternalInput")
    out = nc.dram_tensor("out", [tot_slots, D], bf16, kind="ExternalOutput")

    sp = spans[:]

    def spans_view(run_len):
        # overlapping view [n, run_len*D] with row stride D: index unit is
        # one chunk, each gathered element spans run_len chunks
        if run_len == 1:
            return sp
        return AP(sp.tensor, sp.offset,
                  [[D, N_CHUNKS - run_len + 1], [1, run_len * D]])

    base_of = {}
    acc = 0
    for c in range(len(CLASSES)):
        base_of[c] = acc
        acc += 128 * T[c]

    with TileContext(nc) as tc:
        with (
            tc.tile_pool(name="constp", bufs=1) as const_pool,
            tc.tile_pool(name="outp", bufs=8) as out_pool,
        ):
            # counts first: the gpsimd register load is the longest
            # preamble dependency chain; gidx split so the FIRST gather's
            # indices arrive on a small fast DMA while the bulk streams
            # on the other HWDGE ring
            ncnt_t = const_pool.tile([128, len(GATHERS)], i32)
            nc.sync.dma_start(out=ncnt_t[:], in_=ncnt[:])
            gidx_t = const_pool.tile([128, cols16], i16)
            split16 = min(128 * T[GATHERS[0][0]] // 16, cols16)
            nc.sync.dma_start(out=gidx_t[:, 0:split16],
                              in_=gidx[:, 0:split16])
            nc.scalar.dma_start(out=gidx_t[:, split16:],
                                in_=gidx[:, split16:])

            nc.gpsimd.load_library(library_config.mlp)
            counts = [
                nc.alloc_register(mybir.EngineType.Pool, f"cnt{gi}")
                for gi in range(len(GATHERS))]
            nc.gpsimd.reg_load(counts, ncnt_t[0:1, :])

            land = {}
            for c, shape in enumerate(CLASSES):
                for j, ln in enumerate(shape):
                    if T[c]:
                        land[(c, j)] = const_pool.tile(
                            [128, T[c], ln * D], bf16, name=f"land_{c}_{j}")

            # one gather instruction per (class, slot); per-core real
            # count in num_idxs_reg; queues rotate (each queue_num is a
            # distinct Q7 core pair -> concurrent descriptor generation;
            # the Q7 gather op supports queues 0-2)
            off16 = 0
            qn = 0
            for gi, (c, j, ln) in enumerate(GATHERS):
                if not T[c]:
                    continue
                n_idx = 128 * T[c]
                nc.gpsimd.dma_gather(
                    land[(c, j)][:], spans_view(ln),
                    gidx_t[:, off16:off16 + n_idx // 16],
                    n_idx, counts[gi], ln * D,
                    elem_step=(None if ln == 1 else D),
                    queue_num=qn)
                qn = (qn + 1) % 3
                off16 += n_idx // 16
            assert off16 == cols16, (off16, cols16)

            def tt_max(o, a_, b_):
                nc.vector.tensor_tensor(o, a_, b_, mybir.AluOpType.max)

            # reduce + dense store in class order; stores alternate
            # between the two HWDGE engines (sync / scalar)
            st_eng = [nc.sync, nc.scalar]
            st_i = 0

            def store(dst_rows, src):
                nonlocal st_i
                st_eng[st_i & 1].dma_start(out=dst_rows, in_=src)
                st_i += 1

            def reduce_groups(c, shape, gsl, o):
                """o <- max over the class's chunks for land group slice
                gsl (a slice of the group dim; [128, W, D]-shaped APs)."""
                if shape == (2,):
                    l0 = land[(c, 0)]
                    tt_max(o, l0[:, gsl, 0:D], l0[:, gsl, D:2 * D])
                elif shape == (3,):
                    l0 = land[(c, 0)]
                    tt_max(o, l0[:, gsl, 0:D], l0[:, gsl, D:2 * D])
                    tt_max(o, l0[:, gsl, 2 * D:3 * D], o)
                elif shape == (4,):
                    l0 = land[(c, 0)]
                    tt_max(o, l0[:, gsl, 0:D], l0[:, gsl, D:2 * D])
                    tt_max(o, l0[:, gsl, 2 * D:3 * D], o)
                    tt_max(o, l0[:, gsl, 3 * D:4 * D], o)
                elif shape == (1, 1):
                    tt_max(o, land[(c, 0)][:, gsl, :],
                           land[(c, 1)][:, gsl, :])
                elif shape == (1, 2):
                    l1 = land[(c, 1)]
                    tt_max(o, l1[:, gsl, 0:D], l1[:, gsl, D:2 * D])
                    tt_max(o, land[(c, 0)][:, gsl, :], o)
                else:
                    raise AssertionError(shape)

            def pair_rows_ap(r0):
                # DRAM rows [r0, r0+256) viewed [128, 2*D]: partition p
                # holds rows (r0+2p, r0+2p+1) -> 4KB descriptors
                v = out[r0:r0 + 256, :]
                return AP(v.tensor, v.offset, [[2 * D, 128], [1, 2 * D]])

            for c, shape in enumerate(CLASSES):
                # pair-blocks: groups (2b, 2b+1) hold rows base+b*256+2p+h
                # at (partition p, group-parity h): one [128, 2, D] reduce
                # and one [128, 2*D] store (4KB descriptors, 256 rows)
                for b in range(PB[c]):
                    r0 = base_of[c] + 256 * b
                    gsl = slice(2 * b, 2 * b + 2)
                    if shape == (1,):
                        src = land[(c, 0)][:, gsl, :]
                    else:
                        o = out_pool.tile([128, 2, D], bf16, tag="res")
                        reduce_groups(c, shape, gsl, o[:])
                        src = o[:]
                    store(pair_rows_ap(r0), src)
                # trailing single tiles (per-core pads land here)
                for t in range(2 * PB[c], T[c]):
                    r0 = base_of[c] + 128 * t
                    dst_rows = out[r0:r0 + 128, :]
                    if shape == (1,):
                        store(dst_rows, land[(c, 0)][:, t, :])
                        continue
                    o = out_pool.tile([128, 1, D], bf16, tag="res1")
                    reduce_groups(c, shape, slice(t, t + 1), o[:])
                    store(dst_rows, o[:, 0, :])

    lower_extended_insts(nc)
    _split_multi_wait_instructions(nc)
    _NC_CACHE[T] = nc
    return nc


def _f32_to_bf16_u16(a_f32):
    """Round-to-nearest-even f32 -> bf16 bit pattern (uint16)."""
    u = a_f32.view(np.uint32)
    return ((u + 0x8000 + ((u >> 16) & 1)) >> 16).astype(np.uint16)


def _assign_cores(mask):
    """Deal examples to cores round-robin per pattern: class sizes are
    balanced to +-1 example so the shared (maxed) NEFF pads least."""
    valid = (np.asarray(mask) != 0)
    pat_ex = (valid.astype(np.int64) * (1 << np.arange(S))).sum(1)  # [B]
    ex_of_core = [[] for _ in range(N_CORES)]
    rr = 0
    for p in range(16):
        for e in np.nonzero(pat_ex == p)[0]:
            ex_of_core[rr].append(int(e))
            rr = (rr + 1) % N_CORES
    # equalize totals to B_SH by moving surplus (keeps shard shapes equal)
    surplus = []
    for i in range(N_CORES):
        while len(ex_of_core[i]) > B_SH:
            surplus.append(ex_of_core[i].pop())
    for i in range(N_CORES):
        while len(ex_of_core[i]) < B_SH:
            ex_of_core[i].append(surplus.pop())
    return pat_ex, [np.array(e, np.int64) for e in ex_of_core]


def _core_tables(pat_rows, T, PB):
    """gidx stream + per-instruction counts + (order, slots) maps.
    pat_rows: [ROWS] pattern of each core-local row."""
    sid_rows = _SID[pat_rows]
    live = sid_rows >= 0
    order = np.argsort(
        np.where(live, sid_rows, 10 ** 6), kind="stable")  # k0 rows last
    sid_sorted = np.where(live[order], sid_rows[order], -1)

    per_class_idx = {}
    counts = np.empty(len(GATHERS), np.int32)
    slots = np.full(ROWS, -1, np.int64)
    base = 0
    row_pos = 0
    for c, shape in enumerate(CLASSES):
        rows_c = order[sid_sorted == c]
        n = len(rows_c)
        assert n <= 128 * T[c], (c, n, T[c])
        slots[row_pos:row_pos + n] = base + np.arange(n)
        row_pos += n
        for j in range(len(shape)):
            idx = np.full(128 * T[c], PAD_IDX, np.int64)
            idx[:n] = rows_c * S + _START[j, pat_rows[rows_c]]
            if n == 0 and T[c]:
                idx[0] = 0          # sentinel: >=1 real descriptor
            per_class_idx[(c, j)] = idx
        base += 128 * T[c]
    for gi, (c, j, ln) in enumerate(GATHERS):
        n = int((per_class_idx[(c, j)] >= 0).sum())
        counts[gi] = max(n, 1)

    # pair-region stream permutation: sorted row r = b*256 + 2p + h of
    # the class lands at stream position (2b+h)*128 + p; the trailing
    # singles region keeps the identity layout (r == position)
    segs = []
    for (c, j, ln) in GATHERS:
        idx_lin = per_class_idx[(c, j)]
        seg = idx_lin.copy()
        npair = 256 * PB[c]
        if npair:
            r = np.arange(npair)
            pos = (2 * (r // 256) + (r & 1)) * 128 + (r % 256) // 2
            seg[pos] = idx_lin[r]
        segs.append(seg)
    stream = np.concatenate(segs).astype(np.int16)
    cols16 = len(stream) // 16
    gidx16 = np.zeros((16, cols16), np.int16)
    ppos = np.arange(len(stream))
    gidx16[ppos % 16, ppos // 16] = stream
    gidx = np.tile(gidx16, (8, 1))                     # 8 Q7 cores
    ncnt = np.tile(counts[None, :], (128, 1))
    return gidx, ncnt, order, slots


def _make_all(spans, attention_mask):
    spans = np.asarray(spans)
    mask = np.asarray(attention_mask)
    assert spans.shape == (B, L, S, D), spans.shape
    assert mask.shape == (B, S), mask.shape

    pat_ex, ex_of_core = _assign_cores(mask)
    # per-core class sizes -> shared tile counts
    n_cls = np.zeros((N_CORES, len(CLASSES)), np.int64)
    pat_rows_core = []
    for i in range(N_CORES):
        pr = np.repeat(pat_ex[ex_of_core[i]], L)
        pat_rows_core.append(pr)
        sid = _SID[pr]
        n_cls[i] = np.bincount(sid[sid >= 0], minlength=len(CLASSES))
    T = tuple(int(math.ceil(int(n_cls[:, c].max()) / 128))
              for c in range(len(CLASSES)))
    # full 256-row pair-blocks, all-real on EVERY core (pads would fall
    # mid-stream there and desync the trimmed-count ring contract)
    PB = tuple(min(int(n_cls[:, c].min()) // 256, T[c] // 2)
               for c in range(len(CLASSES)))

    spans_f32 = np.ascontiguousarray(spans, dtype=np.float32)
    spans_rows = spans_f32.reshape(B * L, S * D)

    import ml_dtypes
    in_maps, unperm = [], []
    for i in range(N_CORES):
        gidx, ncnt, order, slots = _core_tables(pat_rows_core[i], T, PB)
        rows_g = (np.repeat(ex_of_core[i] * L, L)
                  + np.tile(np.arange(L), B_SH))      # global row ids
        sp_bf = _f32_to_bf16_u16(
            spans_rows[rows_g]).reshape(N_CHUNKS, D).view(ml_dtypes.bfloat16)
        in_maps.append({"spans": sp_bf, "gidx": gidx, "ncnt": ncnt})
        unperm.append((rows_g, order, slots))
    return T, PB, in_maps, unperm


def run(spans, attention_mask, **spmd_kwargs):
    """Run the device kernel; returns (full_output, BassKernelResults)."""
    T, PB, in_maps, unperm = _make_all(spans, attention_mask)
    nc = _build_nc(T, PB)
    res = run_bass_kernel_spmd(nc, in_maps, core_ids=list(range(N_CORES)),
                               **spmd_kwargs)
    neg_u16 = _f32_to_bf16_u16(np.float32([NEG_FILL]))[0]
    full_u16 = np.empty((B * L, D), np.uint16)
    for i in range(N_CORES):
        rows_g, order, slots = unperm[i]
        out_u16 = res.results[i]["out"].view(np.uint16)
        # sorted position p holds row order[p] in slot slots[p]
        live = slots >= 0
        rows_sorted = rows_g[order]
        full_u16[rows_sorted[live]] = out_u16[slots[live]]
        full_u16[rows_sorted[~live]] = neg_u16        # k=0 rows
    full = (full_u16.astype(np.uint32) << 16).view(np.float32)
    return full.reshape(B, L, D), res


def kernel(spans, attention_mask):
    full, _ = run(spans, attention_mask)
    return full
